# revision 20
# baseline (speedup 1.0000x reference)
"""DSGPM-TP GNN message passing (NNConv + GRU x6 + heads) on 8 TRN2 NeuronCores.

Strategy (SPMD, one program for all cores; per-core behavior comes from data):
  - Edges are owned by the core that owns their dst node (20000/8 = 2500 nodes/core).
  - Per core, edges are sorted by dst and greedily packed into T=64 tiles of 128
    edges with at most A=56 distinct dsts per tile.  The i-th distinct dst of
    tile t is assigned local "slot" A*t+i, so the scatter-add of tile t always
    lands in psum columns [c0(t), c0(t)+256) with c0(t) = (A*t)//128*128 -- a
    compile-time constant schedule shared by all cores.  Nodes with no in-edges
    get spare slots.  SLOTS = A*T = 3584 >= 2500.
  - The per-edge [64,64] conv weight W_e = relu(ea@We1+be1)@We2+be2 is computed
    once on device (PE matmuls) and streamed from HBM (k-major per edge) each
    of the 6 iterations: that stream is the memory roofline.
  - msg[e,:] = x_src[e] @ W_e is computed on VectorE as a broadcast multiply
    [128,(k,h)] followed by a strided reduction over h.
  - gather of x_src: indirect DMA rows from a slot-ordered node table in HBM,
    rebuilt each iteration by an AllGather of each core's updated slot block.
  - scatter-add: one-hot matrices built on device (is_equal vs iota) and a PE
    matmul per tile accumulating into the PSUM agg block; the NNConv root term
    also accumulates into the same PSUM.  GRU runs feature-major so all biases
    are per-partition scalars on the scalar engine.
Host side (numpy) handles sorting/packing/permutation; device handles all
O(E*H*H) work.  Outputs are returned slot-ordered and un-permuted on host.
"""

import numpy as np
from contextlib import ExitStack

import concourse.bass as bass
import concourse.bacc as bacc
import concourse.mybir as mybir
import concourse.tile as tile
from concourse.bass import IndirectOffsetOnAxis
from concourse.masks import make_identity

F32 = mybir.dt.float32
I32 = mybir.dt.int32
AF = mybir.ActivationFunctionType
ALU = mybir.AluOpType


class Cfg:
    def __init__(self, cores=8, n=20000, e=60000, t_tiles=64, a_slots=56):
        self.CORES = cores
        self.N = n
        self.E = e
        self.T = t_tiles
        self.A = a_slots
        self.NPC = n // cores           # nodes per core
        self.SLOTS = self.A * self.T    # local slot count
        self.EPT = 128                  # edges per tile
        self.EPC = self.T * self.EPT    # edge capacity per core
        self.H = 64                     # conv/GRU width (61+3)
        self.HID = 61
        self.EMB = 64
        self.NUM_ATOMS = 24
        self.NUM_CG = 18
        self.ITERS = 6
        self.FEAT = self.EMB + self.NUM_ATOMS + 3  # 91
        assert self.SLOTS >= self.NPC + 1
        assert self.SLOTS % 128 == 0


CFG = Cfg()


# --------------------------------------------------------------------------
# host-side prep
# --------------------------------------------------------------------------

def host_prep(cfg, x, edge_index, edge_attr, extended_feat, embed,
              We1, be1, We2, be2, root, conv_b,
              Wih, Whh, bih, bhh, Wo1, bo1, Wo2, bo2,
              Wc1, bc1, Wc2, bc2):
    N, E, H, A, T = cfg.N, cfg.E, cfg.H, cfg.A, cfg.T
    idx = np.asarray(x)[:, 0].astype(np.int64)
    src = np.asarray(edge_index)[0].astype(np.int64)
    dst = np.asarray(edge_index)[1].astype(np.int64)
    ea = np.asarray(edge_attr, dtype=np.float32)
    ext = np.asarray(extended_feat, dtype=np.float32)

    # initial node features X0 = concat(relu(embed[idx]), ext)  [N, 64]
    X0 = np.concatenate([np.maximum(np.asarray(embed)[idx], 0.0), ext],
                        axis=1).astype(np.float32)

    # ---- pass 1: per-core tiling & slot assignment
    slot_of_node = np.full(N, -1, dtype=np.int64)   # global slot = SLOTS*c + s
    per_core_tiles = []                             # [c] -> list of list of eid
    per_core_nodes_of_slot = []                     # [c] -> [SLOTS] node or -1
    for c in range(cfg.CORES):
        lo, hi = c * cfg.NPC, (c + 1) * cfg.NPC
        em = (dst >= lo) & (dst < hi)
        eids = np.nonzero(em)[0]
        eids = eids[np.argsort(dst[eids], kind="stable")]
        d = dst[eids]
        # group boundaries (runs of equal dst)
        if len(eids):
            starts = np.concatenate([[0], np.nonzero(np.diff(d))[0] + 1])
            ends = np.concatenate([starts[1:], [len(eids)]])
        else:
            starts = ends = np.array([], dtype=np.int64)
        tiles = []
        cur, cur_d = [], 0
        for s0, s1 in zip(starts, ends):
            g = s1 - s0
            assert g <= cfg.EPT, "dst group larger than a tile"
            if len(cur) + g > cfg.EPT or cur_d + 1 > A:
                tiles.append(cur)
                cur, cur_d = [], 0
            cur.extend(eids[s0:s1].tolist())
            cur_d += 1
        if cur:
            tiles.append(cur)
        assert len(tiles) <= T, f"core {c}: {len(tiles)} tiles > {T}"
        tiles += [[] for _ in range(T - len(tiles))]
        nodes_of_slot = np.full(cfg.SLOTS, -1, dtype=np.int64)
        for t, te in enumerate(tiles):
            i = -1
            prev = -1
            for e in te:
                if dst[e] != prev:
                    i += 1
                    prev = dst[e]
                    nodes_of_slot[A * t + i] = dst[e]
                    slot_of_node[dst[e]] = cfg.SLOTS * c + A * t + i
        # spare slots for edge-less nodes
        free = [s for s in range(cfg.SLOTS) if nodes_of_slot[s] < 0]
        fi = 0
        for v in range(lo, hi):
            if slot_of_node[v] < 0:
                s = free[fi]; fi += 1
                nodes_of_slot[s] = v
                slot_of_node[v] = cfg.SLOTS * c + s
        per_core_tiles.append(tiles)
        per_core_nodes_of_slot.append(nodes_of_slot)
    assert (slot_of_node >= 0).all()

    # ---- pass 2: per-core arrays
    per_core = []
    for c in range(cfg.CORES):
        tiles = per_core_tiles[c]
        nodes_of_slot = per_core_nodes_of_slot[c]
        eaT = np.zeros((T, 4, cfg.EPT), dtype=np.float32)
        goff = np.zeros((cfg.EPT, T), dtype=np.int32)
        dstrel = np.full((cfg.EPT, T), -1.0, dtype=np.float32)
        for t, te in enumerate(tiles):
            c0 = (A * t) // 128 * 128
            i = -1
            prev = -1
            for p, e in enumerate(te):
                if dst[e] != prev:
                    i += 1
                    prev = dst[e]
                eaT[t, :, p] = ea[e]
                goff[p, t] = slot_of_node[src[e]]
                dstrel[p, t] = float(A * t + i - c0)
        sl = nodes_of_slot
        valid = sl >= 0
        x0T = np.zeros((H, cfg.SLOTS), dtype=np.float32)
        x0T[:, valid] = X0[sl[valid]].T
        onehotT = np.zeros((cfg.NUM_ATOMS, cfg.SLOTS), dtype=np.float32)
        onehotT[idx[sl[valid]], np.nonzero(valid)[0]] = 1.0
        extT = np.zeros((3, cfg.SLOTS), dtype=np.float32)
        extT[:, valid] = ext[sl[valid]].T
        per_core.append(dict(eaT=eaT, goff=goff, dstrel=dstrel, x0T=x0T,
                             onehotT=onehotT, extT=extT))

    # slot-space initial node table (same for all cores)
    x0all = np.zeros((cfg.CORES * cfg.SLOTS, H), dtype=np.float32)
    for c in range(cfg.CORES):
        sl = per_core_nodes_of_slot[c]
        valid = sl >= 0
        x0all[cfg.SLOTS * c + np.nonzero(valid)[0]] = X0[sl[valid]]

    # ---- shared weights, pre-laid-out
    H2 = H * H
    We2r = np.asarray(We2, dtype=np.float32).reshape(128, H, H)
    shared = dict(
        x0all=x0all,
        We1=np.asarray(We1, dtype=np.float32),
        be1=np.asarray(be1, dtype=np.float32).reshape(128, 1),
        We2T=We2r.transpose(0, 2, 1).reshape(128, H2).copy(),
        be2T=np.asarray(be2, dtype=np.float32).reshape(H, H).T.reshape(1, H2).copy(),
        root=np.asarray(root, dtype=np.float32),
        WihT=np.asarray(Wih, dtype=np.float32).T.copy(),
        WhhT=np.asarray(Whh, dtype=np.float32).T.copy(),
        brz=(np.asarray(bih) + np.asarray(bhh))[:2 * H].astype(np.float32).reshape(2, H).T.copy(),
        b_in=np.asarray(bih, dtype=np.float32)[2 * H:].reshape(H, 1).copy(),
        b_hn=np.asarray(bhh, dtype=np.float32)[2 * H:].reshape(H, 1).copy(),
        convb=np.asarray(conv_b, dtype=np.float32).reshape(H, 1),
        Wo1=np.asarray(Wo1, dtype=np.float32),
        bo1=np.asarray(bo1, dtype=np.float32).reshape(H, 1),
        Wo2=np.asarray(Wo2, dtype=np.float32),
        bo2=np.asarray(bo2, dtype=np.float32).reshape(cfg.EMB, 1),
        Wc1=np.asarray(Wc1, dtype=np.float32),
        bc1=np.asarray(bc1, dtype=np.float32).reshape(2, 128).T.copy(),
        Wc2=np.concatenate([np.asarray(Wc2, dtype=np.float32)[:128],
                            np.asarray(Wc2, dtype=np.float32)[128:]],
                           axis=1).copy(),
        bc2=np.asarray(bc2, dtype=np.float32).reshape(cfg.NUM_CG, 1),
    )
    return per_core, shared, per_core_nodes_of_slot


# --------------------------------------------------------------------------
# device program
# --------------------------------------------------------------------------

def build_program(cfg):
    H, T, A, SLOTS, EPT = cfg.H, cfg.T, cfg.A, cfg.SLOTS, cfg.EPT
    H2 = H * H
    chunks = [(i, min(i + 512, SLOTS)) for i in range(0, SLOTS, 512)]
    WCH = 8 if H2 == 4096 else 1
    wchunks = [(i * (H2 // WCH), (i + 1) * (H2 // WCH)) for i in range(WCH)]

    nc = bacc.Bacc(num_devices=cfg.CORES)

    def ip(name, shape, dtype=F32):
        return nc.declare_dram_parameter(name, list(shape), dtype, isOutput=False)

    p_eaT = ip("eaT", [T, 4, EPT])
    p_goff = ip("goff", [EPT, T], I32)
    p_dstrel = ip("dstrel", [EPT, T])
    p_x0all = ip("x0all", [cfg.CORES * SLOTS, H])
    p_x0T = ip("x0T", [H, SLOTS])
    p_onehotT = ip("onehotT", [cfg.NUM_ATOMS, SLOTS])
    p_extT = ip("extT", [3, SLOTS])
    p_We1 = ip("We1", [4, 128])
    p_be1 = ip("be1", [128, 1])
    p_We2T = ip("We2T", [128, H2])
    p_be2T = ip("be2T", [1, H2])
    p_root = ip("root", [H, H])
    p_WihT = ip("WihT", [H, 3 * H])
    p_WhhT = ip("WhhT", [H, 3 * H])
    p_brz = ip("brz", [H, 2])
    p_bin = ip("b_in", [H, 1])
    p_bhn = ip("b_hn", [H, 1])
    p_convb = ip("convb", [H, 1])
    p_Wo1 = ip("Wo1", [H, H])
    p_bo1 = ip("bo1", [H, 1])
    p_Wo2 = ip("Wo2", [H, cfg.EMB])
    p_bo2 = ip("bo2", [cfg.EMB, 1])
    p_Wc1 = ip("Wc1", [cfg.FEAT, 256])
    p_bc1 = ip("bc1", [128, 2])
    p_Wc2 = ip("Wc2", [128, 2 * cfg.NUM_CG])
    p_bc2 = ip("bc2", [cfg.NUM_CG, 1])

    o_fgT = nc.declare_dram_parameter("fgT", [cfg.FEAT, SLOTS], F32, isOutput=True)
    o_predT = nc.declare_dram_parameter("predT", [cfg.NUM_CG, SLOTS], F32, isOutput=True)

    Whbm = nc.dram_tensor("Whbm", [T, EPT, H2], F32)
    Xlocal = nc.dram_tensor("Xlocal", [SLOTS, H], F32)
    Xshared = nc.dram_tensor("Xshared", [cfg.CORES * SLOTS, H], F32,
                             addr_space="Shared" if cfg.CORES > 4 else "Local")

    with tile.TileContext(nc) as tc, ExitStack() as ctx:
        pool_w = ctx.enter_context(tc.tile_pool(name="wpool", bufs=2))
        pool_c = ctx.enter_context(tc.tile_pool(name="cpool", bufs=2))
        pool_sm = ctx.enter_context(tc.tile_pool(name="smpool", bufs=3))
        pool_st = ctx.enter_context(tc.tile_pool(name="stpool", bufs=1))
        pool_k = ctx.enter_context(tc.tile_pool(name="kpool", bufs=1))
        # PSUM is 16KB/partition total; every psum tile time-multiplexes one slot.
        psum = ctx.enter_context(tc.tile_pool(name="psum", bufs=1, space="PSUM"))

        def sdma(dst_ap, src_ap):
            nc.sync.dma_start(out=dst_ap, in_=src_ap)

        # ---- constants
        def kload(p, shape, dtype=F32, tag=None):
            t_ = pool_k.tile(list(shape), dtype, tag=tag or p.name)
            sdma(t_[:], p[:])
            return t_

        goff_sb = kload(p_goff, [EPT, T], I32)
        dstrel_sb = kload(p_dstrel, [EPT, T])
        We1_sb = kload(p_We1, [4, 128])
        be1_sb = kload(p_be1, [128, 1])
        root_sb = kload(p_root, [H, H])
        WihT_sb = kload(p_WihT, [H, 3 * H])
        WhhT_sb = kload(p_WhhT, [H, 3 * H])
        brz_sb = kload(p_brz, [H, 2])
        bin_sb = kload(p_bin, [H, 1])
        bhn_sb = kload(p_bhn, [H, 1])
        convb_sb = kload(p_convb, [H, 1])
        Wo1_sb = kload(p_Wo1, [H, H])
        bo1_sb = kload(p_bo1, [H, 1])
        Wo2_sb = kload(p_Wo2, [H, cfg.EMB])
        bo2_sb = kload(p_bo2, [cfg.EMB, 1])
        Wc1_sb = kload(p_Wc1, [cfg.FEAT, 256])
        bc1_sb = kload(p_bc1, [128, 2])
        Wc2_sb = kload(p_Wc2, [128, 2 * cfg.NUM_CG])
        bc2_sb = kload(p_bc2, [cfg.NUM_CG, 1])

        ident = pool_k.tile([128, 128], F32, tag="ident")
        make_identity(nc, ident[:])
        iota_i = pool_k.tile([128, 256], I32, tag="iotai")
        nc.gpsimd.iota(iota_i[:], pattern=[[1, 256]], base=0, channel_multiplier=0)
        iota_f = pool_k.tile([128, 256], F32, tag="iotaf")
        nc.vector.tensor_copy(out=iota_f[:], in_=iota_i[:])
        ones1 = pool_k.tile([1, 128], F32, tag="ones1")
        nc.vector.memset(ones1[:], 1.0)
        ones91 = pool_k.tile([cfg.FEAT, 1], F32, tag="ones91")
        nc.vector.memset(ones91[:], 1.0)

        # persistent state: h (feature-major, slot-ordered)
        hT = pool_k.tile([H, SLOTS], F32, tag="hT")
        sdma(hT[:], p_x0T[:])

        # ---- precompute W stream: W_t = relu(ea_t @ We1 + be1) @ We2T' + be2T'
        We2T_sb = pool_c.tile([128, H2], F32, tag="cbuf")
        sdma(We2T_sb[:], p_We2T[:])
        be2T_sb = pool_c.tile([1, H2], F32, tag="cbuf")
        sdma(be2T_sb[:], p_be2T[:])
        for t in range(T):
            ea_t = pool_sm.tile([4, EPT], F32, tag="ea")
            sdma(ea_t[:], p_eaT[t])
            ps_a = psum.tile([128, EPT], F32, tag="big")
            nc.tensor.matmul(ps_a[:], lhsT=We1_sb[:], rhs=ea_t[:],
                             start=True, stop=True)
            aT = pool_sm.tile([128, EPT], F32, tag="aT")
            nc.scalar.activation(aT[:], ps_a[:], AF.Relu, bias=be1_sb[:, 0:1])
            wt = pool_w.tile([128, H2], F32, tag="w")
            for (w0, w1) in wchunks:
                ps_w = psum.tile([128, H2 // WCH], F32, tag="big")
                nc.tensor.matmul(ps_w[:], lhsT=aT[:], rhs=We2T_sb[:, w0:w1],
                                 start=True, stop=False)
                nc.tensor.matmul(ps_w[:], lhsT=ones1[:], rhs=be2T_sb[:, w0:w1],
                                 start=False, stop=True)
                nc.scalar.activation(wt[:, w0:w1], ps_w[:], AF.Copy)
            (nc.sync if t % 2 == 0 else nc.scalar).dma_start(
                out=Whbm[t], in_=wt[:])

        # ---- message-passing iterations
        for it in range(cfg.ITERS):
            Xsrc = p_x0all if it == 0 else Xshared
            agg = psum.tile([H, SLOTS + 128], F32, tag="big")
            nc.vector.memset(agg[:], 0.0)
            for t in range(T):
                wt = pool_w.tile([128, H2], F32, tag="w")
                weng = nc.sync if t % 2 == 0 else nc.scalar
                weng.dma_start(out=wt[:], in_=Whbm[t])
                xg = pool_sm.tile([EPT, H], F32, tag="xg")
                nc.gpsimd.indirect_dma_start(
                    out=xg[:], out_offset=None,
                    in_=Xsrc[:],
                    in_offset=IndirectOffsetOnAxis(ap=goff_sb[:, t:t + 1], axis=0),
                )
                S = pool_sm.tile([EPT, 256], F32, tag="S")
                nc.vector.tensor_tensor(
                    out=S[:],
                    in0=dstrel_sb[:, t:t + 1].to_broadcast([EPT, 256]),
                    in1=iota_f[:],
                    op=ALU.is_equal,
                )
                C = pool_c.tile([128, H2], F32, tag="cbuf")
                c3 = C[:].rearrange("p (k h) -> p k h", h=H)
                nc.vector.tensor_tensor(
                    out=c3,
                    in0=wt[:].rearrange("p (k h) -> p k h", h=H),
                    in1=xg[:].unsqueeze(1).to_broadcast([EPT, H, H]),
                    op=ALU.mult,
                )
                msg = pool_sm.tile([EPT, H], F32, tag="msg")
                nc.vector.tensor_reduce(out=msg[:], in_=c3,
                                        axis=mybir.AxisListType.X, op=ALU.add)
                c0 = (A * t) // 128 * 128
                nc.tensor.matmul(agg[:, c0:c0 + 128], lhsT=msg[:],
                                 rhs=S[:, 0:128], start=False, stop=True,
                                 skip_group_check=True)
                nc.tensor.matmul(agg[:, c0 + 128:c0 + 256], lhsT=msg[:],
                                 rhs=S[:, 128:256], start=False, stop=True,
                                 skip_group_check=True)
            # + out @ root  (feature-major: root.T-free accumulate)
            for (a0, a1) in chunks:
                nc.tensor.matmul(agg[:, a0:a1], lhsT=root_sb[:], rhs=hT[:, a0:a1],
                                 start=False, stop=True, skip_group_check=True)
            m = pool_st.tile([128, SLOTS], F32, tag="sA")
            nc.scalar.activation(m[:H, :], agg[:, 0:SLOTS], AF.Relu,
                                 bias=convb_sb[:, 0:1])
            # GRU gates r and z (separate tiles so every op starts at partition 0)
            r_ps = psum.tile([H, SLOTS], F32, tag="big")
            for (a0, a1) in chunks:
                nc.tensor.matmul(r_ps[:, a0:a1], lhsT=WihT_sb[:, 0:H],
                                 rhs=m[:H, a0:a1], start=True, stop=False)
                nc.tensor.matmul(r_ps[:, a0:a1], lhsT=WhhT_sb[:, 0:H],
                                 rhs=hT[:, a0:a1], start=False, stop=True)
            rg = pool_st.tile([128, SLOTS], F32, tag="sD")
            nc.scalar.activation(rg[:H, :], r_ps[:], AF.Sigmoid, bias=brz_sb[:, 0:1])
            z_ps = psum.tile([H, SLOTS], F32, tag="big")
            for (a0, a1) in chunks:
                nc.tensor.matmul(z_ps[:, a0:a1], lhsT=WihT_sb[:, H:2 * H],
                                 rhs=m[:H, a0:a1], start=True, stop=False)
                nc.tensor.matmul(z_ps[:, a0:a1], lhsT=WhhT_sb[:, H:2 * H],
                                 rhs=hT[:, a0:a1], start=False, stop=True)
            zg = pool_st.tile([128, SLOTS], F32, tag="sE")
            nc.scalar.activation(zg[:H, :], z_ps[:], AF.Sigmoid, bias=brz_sb[:, 1:2])
            # n gate: tanh(Win m + bin + r*(Whn h + bhn))
            gh_ps = psum.tile([H, SLOTS], F32, tag="big")
            for (a0, a1) in chunks:
                nc.tensor.matmul(gh_ps[:, a0:a1], lhsT=WhhT_sb[:, 2 * H:3 * H],
                                 rhs=hT[:, a0:a1], start=True, stop=True)
            gh = pool_st.tile([128, SLOTS], F32, tag="sB")
            nc.scalar.activation(gh[:H, :], gh_ps[:], AF.Identity, bias=bhn_sb[:, 0:1])
            nc.vector.tensor_tensor(out=gh[:H, :], in0=rg[:H, :], in1=gh[:H, :],
                                    op=ALU.mult)
            n1_ps = psum.tile([H, SLOTS], F32, tag="big")
            for (a0, a1) in chunks:
                nc.tensor.matmul(n1_ps[:, a0:a1], lhsT=WihT_sb[:, 2 * H:3 * H],
                                 rhs=m[:H, a0:a1], start=True, stop=True)
            nc.vector.tensor_tensor(out=m[:H, :], in0=n1_ps[:], in1=gh[:H, :],
                                    op=ALU.add)
            nn = pool_st.tile([128, SLOTS], F32, tag="sC")
            nc.scalar.activation(nn[:H, :], m[:H, :], AF.Tanh, bias=bin_sb[:, 0:1])
            # h' = n + z*(h - n)
            nc.vector.tensor_tensor(out=m[:H, :], in0=hT[:], in1=nn[:H, :],
                                    op=ALU.subtract)
            nc.vector.tensor_tensor(out=m[:H, :], in0=zg[:H, :], in1=m[:H, :],
                                    op=ALU.mult)
            nc.vector.tensor_tensor(out=hT[:], in0=nn[:H, :], in1=m[:H, :],
                                    op=ALU.add)
            if it < cfg.ITERS - 1:
                # node table update: transpose h, DMA out, allgather
                xnm = pool_st.tile([128, (SLOTS // 128) * H], F32, tag="xnm")
                for st in range(SLOTS // 128):
                    ps_t = psum.tile([128, H], F32, tag="big")
                    nc.tensor.transpose(out=ps_t[:], in_=hT[:, st * 128:(st + 1) * 128],
                                        identity=ident[0:H, 0:H])
                    nc.scalar.activation(xnm[:, st * H:(st + 1) * H], ps_t[:], AF.Copy)
                sdma(Xlocal[:].rearrange("(s p) k -> p s k", p=128),
                     xnm[:].rearrange("p (s k) -> p s k", k=H))
                nc.gpsimd.collective_compute(
                    "AllGather", ALU.bypass,
                    replica_groups=[list(range(cfg.CORES))],
                    ins=[Xlocal[:]], outs=[Xshared[:]],
                )

        # ---- output heads
        o1_ps = psum.tile([H, SLOTS], F32, tag="big")
        for (a0, a1) in chunks:
            nc.tensor.matmul(o1_ps[:, a0:a1], lhsT=Wo1_sb[:], rhs=hT[:, a0:a1],
                             start=True, stop=True)
        o1 = pool_st.tile([128, SLOTS], F32, tag="sB")
        nc.scalar.activation(o1[:H, :], o1_ps[:], AF.Relu, bias=bo1_sb[:, 0:1])
        feat = pool_st.tile([128, SLOTS], F32, tag="sD")
        sdma(feat[cfg.EMB:cfg.EMB + cfg.NUM_ATOMS, :], p_onehotT[:])
        sdma(feat[cfg.EMB + cfg.NUM_ATOMS:cfg.FEAT, :], p_extT[:])
        e_ps = psum.tile([cfg.EMB, SLOTS], F32, tag="big")
        for (a0, a1) in chunks:
            nc.tensor.matmul(e_ps[:, a0:a1], lhsT=Wo2_sb[:], rhs=o1[:H, a0:a1],
                             start=True, stop=True)
        nc.scalar.activation(feat[0:cfg.EMB, :], e_ps[:], AF.Identity,
                             bias=bo2_sb[:, 0:1])
        sq = pool_st.tile([128, SLOTS], F32, tag="sA")
        nc.scalar.activation(sq[:cfg.FEAT, :], feat[:cfg.FEAT, :], AF.Square)
        ss_ps = psum.tile([1, SLOTS], F32, tag="big")
        for (a0, a1) in chunks:
            nc.tensor.matmul(ss_ps[:, a0:a1], lhsT=ones91[:],
                             rhs=sq[:cfg.FEAT, a0:a1], start=True, stop=True)
        # norm rows reuse row 0 of tiles that are dead by this point
        snorm = hT[0:1, :]
        nc.scalar.activation(snorm, ss_ps[:], AF.Sqrt)
        rnorm = o1[0:1, :]
        nc.vector.reciprocal(rnorm, snorm)
        fgT = pool_st.tile([128, SLOTS], F32, tag="sA")
        for (a0, a1) in chunks:
            rb = psum.tile([cfg.FEAT, a1 - a0], F32, tag="big")
            nc.tensor.matmul(rb[:], lhsT=ones1[0:1, 0:cfg.FEAT],
                             rhs=rnorm[0:1, a0:a1], start=True, stop=True,
                             skip_group_check=True)
            nc.vector.tensor_tensor(out=fgT[:cfg.FEAT, a0:a1],
                                    in0=feat[:cfg.FEAT, a0:a1], in1=rb[:],
                                    op=ALU.mult)
        sdma(o_fgT[:], fgT[:cfg.FEAT, :])
        predT = pool_st.tile([128, SLOTS], F32, tag="sC")
        cw = min(512, SLOTS)
        for (a0, a1) in chunks:
            w = a1 - a0
            pp = psum.tile([128, 3 * cw], F32, tag="big")
            p1a = pp[:, 0:w]
            p1b = pp[:, cw:cw + w]
            p2 = pp[0:cfg.NUM_CG, 2 * cw:2 * cw + w]
            nc.tensor.matmul(p1a, lhsT=Wc1_sb[:, 0:128], rhs=fgT[:cfg.FEAT, a0:a1],
                             start=True, stop=True, skip_group_check=True)
            nc.tensor.matmul(p1b, lhsT=Wc1_sb[:, 128:256], rhs=fgT[:cfg.FEAT, a0:a1],
                             start=True, stop=True, skip_group_check=True)
            c1a = pool_sm.tile([128, cw], F32, tag="c1a")
            c1b = pool_sm.tile([128, cw], F32, tag="c1b")
            nc.scalar.activation(c1a[:, :w], p1a, AF.Relu, bias=bc1_sb[:, 0:1])
            nc.scalar.activation(c1b[:, :w], p1b, AF.Relu, bias=bc1_sb[:, 1:2])
            nc.tensor.matmul(p2, lhsT=Wc2_sb[:, 0:cfg.NUM_CG], rhs=c1a[:, :w],
                             start=True, stop=False, skip_group_check=True)
            nc.tensor.matmul(p2, lhsT=Wc2_sb[:, cfg.NUM_CG:2 * cfg.NUM_CG],
                             rhs=c1b[:, :w], start=False, stop=True,
                             skip_group_check=True)
            nc.scalar.activation(predT[:cfg.NUM_CG, a0:a1], p2, AF.Identity,
                                 bias=bc2_sb[:, 0:1])
        sdma(o_predT[:], predT[:cfg.NUM_CG, :])

    nc.compile()
    return nc


# --------------------------------------------------------------------------
# entry point
# --------------------------------------------------------------------------

_PROGRAM_CACHE = {}


def _get_program(cfg):
    key = (cfg.CORES, cfg.N, cfg.E, cfg.T, cfg.A)
    if key not in _PROGRAM_CACHE:
        _PROGRAM_CACHE[key] = build_program(cfg)
    return _PROGRAM_CACHE[key]


def run(cfg, inputs, runner=None):
    per_core, shared, nodes_of_slot = host_prep(cfg, **inputs)
    nc = _get_program(cfg)
    in_maps = []
    for c in range(cfg.CORES):
        im = dict(shared)
        im.update(per_core[c])
        in_maps.append(im)
    if runner is None:
        from concourse.bass_utils import run_bass_kernel_spmd
        res = run_bass_kernel_spmd(nc, in_maps, list(range(cfg.CORES))).results
    else:
        res = runner(nc, in_maps)
    fg = np.zeros((cfg.N, cfg.FEAT), dtype=np.float32)
    pred = np.zeros((cfg.N, cfg.NUM_CG), dtype=np.float32)
    for c in range(cfg.CORES):
        sl = nodes_of_slot[c]
        valid = sl >= 0
        fg[sl[valid]] = res[c]["fgT"][:, valid].T
        pred[sl[valid]] = res[c]["predT"][:, valid].T
    return fg, pred


def kernel(**inputs):
    return run(CFG, inputs)
